# revision 9
# baseline (speedup 1.0000x reference)
"""Trainium2 Bass kernel for a dense transformer block (pre-LN, MHA + GELU MLP).

Sharding: data-parallel over batch — B=8 batch elements map 1:1 onto the 8
NeuronCores; no collectives. Each core runs an identical SPMD program on its
own [1024, 768] slice.

Per-core dataflow (P=128 partitions):
  x normal [tok, d] --LN1 stats (bn_stats)--> xc=(x-mu)*rstd (fused DVE)
    --PE transpose--> xhatT [d, tok] bf16
  QKV: qT/kT = W.T @ xhatT (weights stationary, transposed out, bias fused in
    PSUM->SBUF copy); v = xhatT.T @ Wv (normal layout) packed as [tok, 12, 65]
    with a ones column per head for the softmax row-sums.
  Attention per head-pair (rows 0-63 / 64-127 share PE row/col groups):
    scoresT[j,i] = khT.T@qhT (K=64, row-packed pair), exp on ACT (no max
    subtraction: |s|<9 is fp32-safe), ctxU^T = v.T@expT col-packed pair into
    one PSUM bank, row-sums via ones-lhsT matmuls into partitions 0/64,
    normalize with reciprocal_approx_fast + partition-broadcast DMA, fused
    into the PSUM->SBUF copy.
  Wo: attn = ctxT.T @ Wo (normal out) + x residual fused; += bo' on GpSimd.
  LN2 same as LN1 -> x2hatT; fc1 = W1.T @ x2hatT, bias+GELU fused on ACT;
  fc2 = gT.T @ W2 (normal out) + x2 residual fused -> out.

Host-side folds (exact algebra, no approximation):
  Wq' = diag(g1)Wq/8, bq' = (b1@Wq+bq)/8; Wk',bk' same (unscaled); Wv' no
  bias (bv' folded: bo' = bo + (b1@Wv+bv)@Wo); W1' = diag(g2)W1,
  b1' = b2ln@W1+b1. Weights cast to bf16 on host; f32 accumulation on PE.
"""

import numpy as np
import ml_dtypes

import concourse.bass as bass
import concourse.mybir as mybir
from concourse import bacc
from concourse.tile import TileContext
from concourse.masks import make_identity
from concourse.bass_utils import run_bass_kernel_spmd

f32 = mybir.dt.float32
bf16 = mybir.dt.bfloat16
AF = mybir.ActivationFunctionType
ALU = mybir.AluOpType
ts = bass.ts

B = 8
N = 1024
D = 768
H = 12
DH = 64
FF = 3072
EPS = 1e-6
P = 128
NT = N // P    # 8 token tiles
DT = D // P    # 6 d tiles
FT = FF // P   # 24 ff tiles
CW = 512       # free-dim chunk (one PSUM bank of fp32)
NC_CHUNKS = N // CW  # 2
NCORES = 8

_PROGRAM = None
_TAPS = frozenset()
_tap_handles = {}


def _tap(nc, name, aps):
    if name not in _TAPS:
        return
    shape = [len(aps)] + list(aps[0].shape)
    dt = aps[0].dtype
    h = nc.declare_dram_parameter(f"dbg_{name}", shape, dt, True)
    _tap_handles[f"dbg_{name}"] = shape
    for i, ap in enumerate(aps):
        nc.sync.dma_start(out=h[i], in_=ap)


def _bcast_ap(ap_row, parts):
    """AP reading one partition row broadcast across `parts` partitions."""
    return bass.AP(tensor=ap_row.tensor, offset=ap_row.offset,
                   ap=[[0, parts]] + list(ap_row.ap[1:]))


def _layernorm_to_transposed(nc, tc, pools, x_tiles, xhatT, eps_t, ident, tag):
    """LN stats + center/scale in normal layout, then PE-transpose into
    bf16 [d, tok] tiles."""
    ln_pool, xc_pool, ps_tr = pools
    for t in range(NT):
        stats = ln_pool.tile([P, 3, 6], f32, tag=f"{tag}stats", name=f"{tag}stats")
        for s3 in range(3):
            nc.vector.bn_stats(out=stats[:, s3, :],
                               in_=x_tiles[t][:, s3 * 256:(s3 + 1) * 256])
        mv = ln_pool.tile([P, 2], f32, tag=f"{tag}mv", name=f"{tag}mv")
        nc.vector.bn_aggr(out=mv, in_=stats)
        std = ln_pool.tile([P, 1], f32, tag=f"{tag}std", name=f"{tag}std")
        nc.scalar.activation(out=std, in_=mv[:, 1:2], func=AF.Sqrt,
                             bias=eps_t[:, 0:1])
        rstd = ln_pool.tile([P, 1], f32, tag=f"{tag}rstd", name=f"{tag}rstd")
        nc.vector.reciprocal(out=rstd, in_=std)
        xc = xc_pool.tile([P, D], f32, tag=f"{tag}xc", name=f"{tag}xc")
        nc.vector.tensor_scalar(out=xc, in0=x_tiles[t], scalar1=mv[:, 0:1],
                                scalar2=rstd, op0=ALU.subtract, op1=ALU.mult)
        for j in range(DT):
            tp = ps_tr.tile([P, P], f32, tag=f"{tag}tr", name=f"{tag}tr")
            nc.tensor.transpose(tp, xc[:, ts(j, P)], ident)
            if (t * DT + j) % 2 == 0:
                nc.vector.tensor_copy(out=xhatT[j][:, ts(t, P)], in_=tp)
            else:
                nc.scalar.copy(out=xhatT[j][:, ts(t, P)], in_=tp)


def _build_program():
    nc = bacc.Bacc("TRN2", target_bir_lowering=False, debug=False,
                   num_devices=NCORES)

    xd = nc.declare_dram_parameter("x", [N, D], f32, False)
    wqd = nc.declare_dram_parameter("wq", [D, D], bf16, False)
    wkd = nc.declare_dram_parameter("wk", [D, D], bf16, False)
    wvd = nc.declare_dram_parameter("wv", [D, D], bf16, False)
    wod = nc.declare_dram_parameter("wo", [D, D], bf16, False)
    w1d = nc.declare_dram_parameter("w1", [D, FF], bf16, False)
    w2d = nc.declare_dram_parameter("w2", [FF, D], bf16, False)
    bqd = nc.declare_dram_parameter("bq", [P, DT], f32, False)
    bkd = nc.declare_dram_parameter("bk", [P, DT], f32, False)
    b1d = nc.declare_dram_parameter("b1", [P, FT], f32, False)
    bobd = nc.declare_dram_parameter("bob", [P, D], f32, False)
    b2bd = nc.declare_dram_parameter("b2b", [P, D], f32, False)
    outd = nc.declare_dram_parameter("out", [N, D], f32, True)

    with TileContext(nc) as tc:
        _emit_body(nc, tc, xd, wqd, wkd, wvd, wod, w1d, w2d,
                   bqd, bkd, b1d, bobd, b2bd, outd)
    nc.compile()
    return nc


def _emit_body(nc, tc, xd, wqd, wkd, wvd, wod, w1d, w2d,
               bqd, bkd, b1d, bobd, b2bd, outd):
    with tc.tile_pool(name="const", bufs=1) as constp, \
         tc.tile_pool(name="persist", bufs=1) as persist:
        ident = constp.tile([P, P], f32)
        make_identity(nc, ident)
        eps_t = constp.tile([P, 1], f32)
        nc.vector.memset(eps_t, EPS)
        ones1 = constp.tile([P, 1], bf16)
        nc.vector.memset(ones1, 1.0)
        bq_sb = constp.tile([P, DT], f32)
        nc.sync.dma_start(out=bq_sb, in_=bqd[:, :])
        bk_sb = constp.tile([P, DT], f32)
        nc.sync.dma_start(out=bk_sb, in_=bkd[:, :])
        b1_sb = constp.tile([P, FT], f32)
        nc.sync.dma_start(out=b1_sb, in_=b1d[:, :])
        bo_b = constp.tile([P, D], f32)
        nc.sync.dma_start(out=bo_b, in_=bobd[:, :])
        b2_b = constp.tile([P, D], f32)
        nc.sync.dma_start(out=b2_b, in_=b2bd[:, :])

        x_sb = [persist.tile([P, D], f32, tag=f"x{t}", name=f"x{t}") for t in range(NT)]
        for t in range(NT):
            nc.sync.dma_start(out=x_sb[t], in_=xd[ts(t, P), :])
        x2_sb = [persist.tile([P, D], f32, tag=f"x2_{t}", name=f"x2_{t}") for t in range(NT)]

        # ---------- Phase 1: LN1 + transpose ----------
        with tc.tile_pool(name="xhatT", bufs=1) as xhatp, \
             tc.tile_pool(name="qkv", bufs=1) as qkvp:
            xhatT = [xhatp.tile([P, N], bf16, tag=f"xh{j}", name=f"xh{j}") for j in range(DT)]
            with tc.tile_pool(name="ln1", bufs=3) as ln_pool, \
                 tc.tile_pool(name="xc1", bufs=3) as xc_pool, \
                 tc.tile_pool(name="ps_tr1", bufs=3, space="PSUM") as ps_tr:
                _layernorm_to_transposed(nc, tc, (ln_pool, xc_pool, ps_tr),
                                         x_sb, xhatT, eps_t, ident, "l1")

            _tap(nc, "xh", xhatT)
            # ---------- Phase 2: QKV projections ----------
            qT = [qkvp.tile([P, N], bf16, tag=f"q{m}", name=f"q{m}") for m in range(DT)]
            kT = [qkvp.tile([P, N], bf16, tag=f"k{m}", name=f"k{m}") for m in range(DT)]
            v3 = [qkvp.tile([P, H, DH + 1], bf16, tag=f"v{t}", name=f"v{t}")
                  for t in range(NT)]
            with tc.tile_pool(name="wqkv", bufs=1) as wp, \
                 tc.tile_pool(name="ps_qkv", bufs=2, space="PSUM") as psq:
                wq_sb = [wp.tile([P, D], bf16, tag=f"wq{j}", name=f"wq{j}") for j in range(DT)]
                wk_sb = [wp.tile([P, D], bf16, tag=f"wk{j}", name=f"wk{j}") for j in range(DT)]
                wv_sb = [wp.tile([P, D], bf16, tag=f"wv{j}", name=f"wv{j}") for j in range(DT)]
                for j in range(DT):
                    nc.sync.dma_start(out=wq_sb[j], in_=wqd[ts(j, P), :])
                    nc.sync.dma_start(out=wk_sb[j], in_=wkd[ts(j, P), :])
                    nc.sync.dma_start(out=wv_sb[j], in_=wvd[ts(j, P), :])
                for m in range(DT):
                    for c in range(NC_CHUNKS):
                        ps = psq.tile([P, CW], f32, tag="qps", name="qps")
                        for j in range(DT):
                            nc.tensor.matmul(ps, wq_sb[j][:, ts(m, P)],
                                             xhatT[j][:, ts(c, CW)],
                                             start=(j == 0), stop=(j == DT - 1))
                        nc.vector.tensor_scalar_add(
                            out=qT[m][:, ts(c, CW)], in0=ps,
                            scalar1=bq_sb[:, m:m + 1])
                        ps = psq.tile([P, CW], f32, tag="kps", name="kps")
                        for j in range(DT):
                            nc.tensor.matmul(ps, wk_sb[j][:, ts(m, P)],
                                             xhatT[j][:, ts(c, CW)],
                                             start=(j == 0), stop=(j == DT - 1))
                        nc.scalar.activation(out=kT[m][:, ts(c, CW)], in_=ps,
                                             func=AF.Identity,
                                             bias=bk_sb[:, m:m + 1])
                for t in range(NT):
                    nc.vector.memset(v3[t][:, :, DH:DH + 1], 1.0)
                    for lo, w in ((0, 512), (512, 256)):
                        ps = psq.tile([P, 512], f32, tag="vps", name="vps")
                        for j in range(DT):
                            nc.tensor.matmul(ps[:, 0:w],
                                             xhatT[j][:, ts(t, P)],
                                             wv_sb[j][:, lo:lo + w],
                                             start=(j == 0), stop=(j == DT - 1))
                        h0, nh = lo // DH, w // DH
                        nc.vector.tensor_copy(
                            out=v3[t][:, h0:h0 + nh, 0:DH],
                            in_=ps[:, 0:w].rearrange("p (h d) -> p h d", d=DH))

            _tap(nc, "q", qT)
            _tap(nc, "k", kT)
            _tap(nc, "v", v3)
            # ---------- Phase 3: attention (head pairs) ----------
            ctxT = [qkvp.tile([P, N], bf16, tag=f"ctx{m}", name=f"ctx{m}") for m in range(DT)]
            with tc.tile_pool(name="expp", bufs=2) as expp, \
                 tc.tile_pool(name="smallp", bufs=3) as smp, \
                 tc.tile_pool(name="dramp", bufs=3, space="DRAM") as drp, \
                 tc.tile_pool(name="ps_s", bufs=2, space="PSUM") as ps_s, \
                 tc.tile_pool(name="ps_ctx", bufs=2, space="PSUM") as ps_ctx:
                for hp in range(H // 2):
                    h0, h1 = 2 * hp, 2 * hp + 1
                    for c in range(NC_CHUNKS):
                        e0 = expp.tile([P, NT, CW], bf16, tag="e0", name="e0")
                        e1 = expp.tile([P, NT, CW], bf16, tag="e1", name="e1")
                        cps = ps_ctx.tile([P, CW], f32, tag="ctx", name="ctxps")
                        rps = ps_ctx.tile([P, CW], f32, tag="rs", name="rsps")
                        for j in range(NT):
                            s0 = ps_s.tile([P, CW], f32, tag="s0", name="s0")
                            nc.tensor.matmul(s0, kT[hp][0:DH, ts(j, P)],
                                             qT[hp][0:DH, ts(c, CW)],
                                             start=True, stop=True)
                            s1 = ps_s.tile([P, CW], f32, tag="s1", name="s1")
                            nc.tensor.matmul(s1, kT[hp][DH:P, ts(j, P)],
                                             qT[hp][DH:P, ts(c, CW)],
                                             start=True, stop=True)
                            nc.scalar.activation(out=e0[:, j, :], in_=s0,
                                                 func=AF.Exp)
                            nc.scalar.activation(out=e1[:, j, :], in_=s1,
                                                 func=AF.Exp)
                            first, last = j == 0, j == NT - 1
                            nc.tensor.matmul(cps[0:DH, :], v3[j][:, h0, 0:DH],
                                             e0[:, j, :], start=first,
                                             stop=last, tile_position=(0, 0))
                            nc.tensor.matmul(cps[DH:P, :], v3[j][:, h1, 0:DH],
                                             e1[:, j, :], start=first,
                                             stop=last, tile_position=(0, 64))
                            nc.tensor.matmul(rps[0:1, :], ones1, e0[:, j, :],
                                             start=first, stop=last,
                                             tile_position=(0, 0))
                            nc.tensor.matmul(rps[DH:DH + 1, :], ones1,
                                             e1[:, j, :], start=first,
                                             stop=last, tile_position=(0, 64))
                        rec = smp.tile([P, CW], f32, tag="rec", name="rec")
                        nc.vector.reciprocal_approx_fast(out=rec[0:1, :],
                                                         in_=rps[0:1, :])
                        # reciprocal_approx_fast mis-executes on partition-
                        # base-64 slices (constant output); use the exact op.
                        nc.vector.reciprocal(out=rec[DH:DH + 1, :],
                                             in_=rps[DH:DH + 1, :])
                        rb = smp.tile([P, CW], f32, tag="rb", name="rb")
                        nc.gpsimd.partition_broadcast(rb[0:DH, :], rec[0:1, :])
                        # partition_broadcast reads absolute partition 0 only;
                        # bounce the h1 recip row through DRAM to broadcast it.
                        drow = drp.tile([1, CW], f32, tag="drow", name="drow")
                        nc.sync.dma_start(out=drow, in_=rec[DH:DH + 1, :])
                        nc.sync.dma_start(out=rb[DH:P, :],
                                          in_=_bcast_ap(drow[0:1, :], DH))
                        nc.vector.tensor_mul(ctxT[hp][:, ts(c, CW)], cps, rb)
                        if hp == 0 and c == 0 and "att" in _TAPS:
                            dbg = smp.tile([P, CW], f32, tag="dbgc",
                                           name="dbgc")
                            nc.vector.tensor_copy(out=dbg, in_=cps)
                            dbr = smp.tile([P, CW], f32, tag="dbgr",
                                           name="dbgr")
                            nc.vector.tensor_copy(out=dbr, in_=rps)
                            _tap(nc, "e0", [e0])
                            _tap(nc, "e1", [e1])
                            _tap(nc, "cps", [dbg])
                            _tap(nc, "rps", [dbr])
                            _tap(nc, "rb", [rb])
                            dbrec = smp.tile([P, CW], f32, tag="dbgrec",
                                             name="dbgrec")
                            nc.vector.tensor_copy(out=dbrec, in_=rec)
                            _tap(nc, "rec", [dbrec])
                            dbrec2 = smp.tile([P, CW], f32, tag="dbgrec2",
                                              name="dbgrec2")
                            nc.vector.reciprocal(out=dbrec2[DH:DH + 1, :],
                                                 in_=rps[DH:DH + 1, :])
                            _tap(nc, "rec2", [dbrec2])

            _tap(nc, "ctx", ctxT)
            # ---------- Phase 4: Wo + residual ----------
            with tc.tile_pool(name="wo", bufs=1) as wop, \
                 tc.tile_pool(name="ps_o", bufs=2, space="PSUM") as ps_o:
                wo_sb = [wop.tile([P, D], bf16, tag=f"wo{j}", name=f"wo{j}") for j in range(DT)]
                for j in range(DT):
                    nc.sync.dma_start(out=wo_sb[j], in_=wod[ts(j, P), :])
                for t in range(NT):
                    for lo, w in ((0, 512), (512, 256)):
                        ps = ps_o.tile([P, 512], f32, tag="ops", name="ops")
                        for j in range(DT):
                            nc.tensor.matmul(ps[:, 0:w], ctxT[j][:, ts(t, P)],
                                             wo_sb[j][:, lo:lo + w],
                                             start=(j == 0), stop=(j == DT - 1))
                        nc.vector.scalar_tensor_tensor(
                            out=x2_sb[t][:, lo:lo + w], in0=ps[:, 0:w],
                            scalar=1.0, in1=x_sb[t][:, lo:lo + w],
                            op0=ALU.mult, op1=ALU.add)
                    nc.gpsimd.tensor_add(out=x2_sb[t], in0=x2_sb[t], in1=bo_b)

        _tap(nc, "x2", x2_sb)
        # ---------- Phase 5: LN2 + transpose ----------
        with tc.tile_pool(name="x2hatT", bufs=1) as x2hatp:
            x2hatT = [x2hatp.tile([P, N], bf16, tag=f"x2h{j}", name=f"x2h{j}")
                      for j in range(DT)]
            with tc.tile_pool(name="ln2", bufs=3) as ln_pool, \
                 tc.tile_pool(name="xc2", bufs=3) as xc_pool, \
                 tc.tile_pool(name="ps_tr2", bufs=3, space="PSUM") as ps_tr:
                _layernorm_to_transposed(nc, tc, (ln_pool, xc_pool, ps_tr),
                                         x2_sb, x2hatT, eps_t, ident, "l2")
            for t in range(NT):
                nc.gpsimd.tensor_add(out=x2_sb[t], in0=x2_sb[t], in1=b2_b)

            # ---------- Phase 6: MLP ----------
            with tc.tile_pool(name="gT", bufs=1) as gp, \
                 tc.tile_pool(name="w1p", bufs=3) as w1p, \
                 tc.tile_pool(name="w2p", bufs=1) as w2p, \
                 tc.tile_pool(name="outp", bufs=3) as outp, \
                 tc.tile_pool(name="ps_f1", bufs=3, space="PSUM") as ps_f1, \
                 tc.tile_pool(name="ps_f2", bufs=2, space="PSUM") as ps_f2:
                gT = [gp.tile([P, N], bf16, tag=f"g{m}", name=f"g{m}") for m in range(FT)]
                w2_sb = [w2p.tile([P, D], bf16, tag=f"w2_{m}", name=f"w2_{m}")
                         for m in range(FT)]
                for m in range(FT):
                    nc.sync.dma_start(out=w2_sb[m], in_=w2d[ts(m, P), :])
                for m in range(FT):
                    w1m = w1p.tile([P, DT, P], bf16, tag="w1m", name="w1m")
                    nc.sync.dma_start(
                        out=w1m,
                        in_=w1d[:, ts(m, P)].rearrange("(jt p) f -> p jt f",
                                                       p=P))
                    for c in range(NC_CHUNKS):
                        ps = ps_f1.tile([P, CW], f32, tag="f1", name="f1")
                        for j in range(DT):
                            nc.tensor.matmul(ps, w1m[:, j, :],
                                             x2hatT[j][:, ts(c, CW)],
                                             start=(j == 0), stop=(j == DT - 1))
                        nc.scalar.activation(out=gT[m][:, ts(c, CW)], in_=ps,
                                             func=AF.Gelu,
                                             bias=b1_sb[:, m:m + 1])
                _tap(nc, "x2h", x2hatT)
                _tap(nc, "g", gT)
                for t in range(NT):
                    ot = outp.tile([P, D], f32, tag="out", name="outt")
                    for lo, w in ((0, 512), (512, 256)):
                        ps = ps_f2.tile([P, 512], f32, tag="f2", name="f2")
                        for m in range(FT):
                            nc.tensor.matmul(ps[:, 0:w], gT[m][:, ts(t, P)],
                                             w2_sb[m][:, lo:lo + w],
                                             start=(m == 0), stop=(m == FT - 1))
                        nc.vector.scalar_tensor_tensor(
                            out=ot[:, lo:lo + w], in0=ps[:, 0:w], scalar=1.0,
                            in1=x2_sb[t][:, lo:lo + w],
                            op0=ALU.mult, op1=ALU.add)
                    nc.sync.dma_start(out=outd[ts(t, P), :], in_=ot)


def _get_program():
    global _PROGRAM
    if _PROGRAM is None:
        _PROGRAM = _build_program()
    return _PROGRAM


def _prepare_host_inputs(inputs):
    f64 = np.float64
    x = np.asarray(inputs["x"], np.float32)
    g1 = np.asarray(inputs["ln1_g"], f64)
    b1l = np.asarray(inputs["ln1_b"], f64)
    g2 = np.asarray(inputs["ln2_g"], f64)
    b2l = np.asarray(inputs["ln2_b"], f64)
    Wq = np.asarray(inputs["Wq"], f64)
    Wk = np.asarray(inputs["Wk"], f64)
    Wv = np.asarray(inputs["Wv"], f64)
    Wo = np.asarray(inputs["Wo"], f64)
    W1 = np.asarray(inputs["W1"], f64)
    W2 = np.asarray(inputs["W2"], f64)
    bq = np.asarray(inputs["bq"], f64)
    bk = np.asarray(inputs["bk"], f64)
    bv = np.asarray(inputs["bv"], f64)
    bo = np.asarray(inputs["bo"], f64)
    b1 = np.asarray(inputs["b1"], f64)
    b2 = np.asarray(inputs["b2"], f64)

    def bf(a):
        return np.ascontiguousarray(a.astype(np.float32)).astype(
            ml_dtypes.bfloat16)

    def col_tile(vec, nt):  # [nt*P] -> [P, nt] (partition-major per tile)
        return np.ascontiguousarray(
            vec.astype(np.float32).reshape(nt, P).T)

    wq_h = bf(g1[:, None] * Wq * 0.125)
    bq_h = col_tile((b1l @ Wq + bq) * 0.125, DT)
    wk_h = bf(g1[:, None] * Wk)
    bk_h = col_tile(b1l @ Wk + bk, DT)
    wv_h = bf(g1[:, None] * Wv)
    bv_f = b1l @ Wv + bv
    wo_h = bf(Wo)
    bo_f = bo + bv_f @ Wo
    bob_h = np.ascontiguousarray(
        np.broadcast_to(bo_f.astype(np.float32), (P, D)))
    w1_h = bf(g2[:, None] * W1)
    b1_h = col_tile(b2l @ W1 + b1, FT)
    w2_h = bf(W2)
    b2b_h = np.ascontiguousarray(
        np.broadcast_to(b2.astype(np.float32), (P, D)))

    shared = {"wq": wq_h, "wk": wk_h, "wv": wv_h, "wo": wo_h,
              "w1": w1_h, "w2": w2_h, "bq": bq_h, "bk": bk_h,
              "b1": b1_h, "bob": bob_h, "b2b": b2b_h}
    return x, shared


def kernel(**inputs):
    x, shared = _prepare_host_inputs(inputs)
    nc = _get_program()
    in_maps = [dict(shared, x=np.ascontiguousarray(x[c]))
               for c in range(NCORES)]
    import time
    t0 = time.perf_counter()
    res = run_bass_kernel_spmd(nc, in_maps, list(range(NCORES)))
    t1 = time.perf_counter()
    kernel._last_wall_s = t1 - t0
    out = np.stack([res.results[c]["out"] for c in range(NCORES)], axis=0)
    return out.astype(np.float32)
